# revision 19
# baseline (speedup 1.0000x reference)
"""Multi-head graph attention (GAT) Trainium2 kernel, 8-core SPMD.

Problem: h[4096,256], adj[4096,4096] bool, w[4,256,64], a_src/a_dst[4,64,1],
bias[64] -> out[4096,4,64]:
    h_prime = h @ w[k]                       per head
    s[i,j]  = src[i] + dst[j]                (rank-1!)
    scores  = leaky_relu(s, 0.2), masked by adj, softmax over j
    out     = attn @ h_prime + bias

Sharding: 8 cores = 2 head-groups x 4 row-blocks. Core c computes heads
[2*(c%2), 2*(c%2)+1] for output rows [1024*(c//2), 1024*(c//2)+1024).

Key algebra (all on-device, per head):
    exp(leaky(s)) = e^{0.2 s} * max(e^{0.8 s}, 1);  e^{0.8 s} = esrc_i*edst_j
The per-column factor e^{0.2 src_i} cancels in the softmax normalization,
so the unnormalized weights reduce to
    P'[j,i] = e^{0.2 dst_j} * adj[i,j] * max(e^{0.8 src_i} * e^{0.8 dst_j}, 1)
The e^{0.2 dst_j} row-factor rides the stationary G operand of the final
bmm (col O of G holds the factor itself -> softmax denominator row).

v2.1 structure (DVE is the wall; measured: GpSimd software tensor ops are
~10x the cost model -- useless for bulk work; stride-0 broadcast TT also
falls off the DVE fast path):
  - classic chunks: r = max(esrc*edst, 1) via DVE tensor_scalar, then
    p = r * adjT via DVE tensor_tensor, per head.
  - RELU_CHUNKS use max(x,1)*adj = relu(x-1)*adj + adj: the relu runs on
    the otherwise-slack ACT engine, and the "+adj" term is two extra PE
    matmuls accumulating bmm(G, adjT-chunk) into the same psum acc --
    issued inside hp_block, ahead of the DVE, where the PE is idle.
  - host folds v = w @ a (parameter folding) and pre-scales the dst
    columns by 0.8/0.2 so ONE exp covers all 4 exp scalars per chunk.
  - epilogue: accumulators [65,512] are copied (ACT) to bf16 and DMA'd
    raw; the num/den division, transpose, and bias happen on HOST during
    unsharding (pure layout/normalization of the device-computed sums).
"""

import sys

sys.path.insert(0, "/opt/trn_rl_repo")

import numpy as np
import ml_dtypes

N = 4096          # nodes
F = 256           # f_in
O = 64            # f_out
NHEAD = 4
NCORES = 8
NH = 2            # heads per core
NI = 1024         # output rows per core
NCJ = N // 128    # 32 j-chunks
NSEG = NI // 512  # 2 segments of 512 in the i (free) dim
NF = N - NI       # 3072 j-columns outside this core's i-block
NCI = NI // 128   # 8 j-chunks whose h columns come from the hTi tile
WC = NH * O + 2 * NH  # wall cols: [hp h0 | hp h1 | 0.8*vdst | 0.2*vdst]

# chunks using the relu decomposition: max(x,1)*adj = relu(x-1)*adj + adj.
# The relu runs on ACT (off the DVE critical path) and the "+adj" term is
# two extra PE matmuls accumulating bmm(G, adjT_chunk) into the same psum.
RELU_CHUNKS = frozenset({1, 7, 13, 19, 25})
# (chunk, head) mask-multiplies offloaded to the GpSimd Multiply path
# (measured 2.16us per [128,1024] op vs 0.69us DVE; 16 singles balance
# DVE ~53us vs GpSimd ~35us)
GP_TT = frozenset((c, 1) for c in range(NCJ) if c % 2 == 0)

_CACHE = {}


def _build():
    import concourse.bass as bass
    import concourse.bacc as bacc
    import concourse.mybir as mybir
    import concourse.tile as tile
    from concourse.bass import ts

    f32 = mybir.dt.float32
    bf16 = mybir.dt.bfloat16
    Alu = mybir.AluOpType
    Act = mybir.ActivationFunctionType

    nc = bacc.Bacc()
    hTf_d = nc.declare_dram_parameter("hTf", [F, NF], bf16, isOutput=False)
    hTi_d = nc.declare_dram_parameter("hTi", [F, NI], bf16, isOutput=False)
    adjT_d = nc.declare_dram_parameter("adjT", [8 * 128, 4 * NI], bf16, isOutput=False)
    # wall = [w (head-major) | 0.8*v_dst | 0.2*v_dst], v = w @ a folded on
    # host, exponent scales pre-folded so ONE exp op covers all 4 columns
    wall_d = nc.declare_dram_parameter("wall", [F, WC], bf16, isOutput=False)
    vsrc_d = nc.declare_dram_parameter("vsrc", [F, NH], bf16, isOutput=False)
    # raw accumulator output: [head, O+1 rows (num | den), i] in bf16
    out_d = nc.declare_dram_parameter("out", [NH, O + 1, NI], bf16, isOutput=True)

    with tile.TileContext(nc) as tc:
        with (
            tc.tile_pool(name="sb", bufs=1) as sb,
            tc.tile_pool(name="sbr", bufs=2) as sbr,
            tc.tile_pool(name="sbo", bufs=2) as sbo,
            tc.tile_pool(name="pw", bufs=4, space="PSUM") as pw,
            tc.tile_pool(name="pacc", bufs=1, space="PSUM") as pacc,
        ):
            # ---- static SBUF tensors ----
            hT_sb = sb.tile([128, 2, NF], bf16, name="hT_sb")
            hTi_sb = sb.tile([128, 2, NI], bf16, name="hTi_sb")
            adjT_sb = sb.tile([128, NCJ, NI], bf16, name="adjT_sb")
            wall_sb = sb.tile([128, 2, WC], bf16, name="wall_sb")
            vsrc_sb = sb.tile([128, 2, NH], bf16, name="vsrc_sb")
            ones_row = sb.tile([1, 128], bf16, name="ones_row")
            esrc3_rep = sb.tile([128, NH, NI], bf16, name="esrc3_rep")
            esrc3_row = sb.tile([1, NH, NI], bf16, name="esrc3_row")
            g_sb = sb.tile([128, NH, NCJ, O + 1], bf16, name="g_sb")
            # cols 0-1: e^{0.8 dst} (TS/relu scalar), cols 2-3: e^{0.2 dst}
            edst_sb = sb.tile([128, NCJ, 4], f32, name="edst_sb")
            zerob = sb.tile([128, 1], f32, name="zerob")
            nc.vector.memset(zerob[:, :], 0.0)
            negone = sb.tile([128, 1], f32, name="negone")
            nc.vector.memset(negone[:, :], -1.0)
            # NOTE: do NOT add an exp-table warm-up op here -- tested three
            # times in the baseline, always slower.

            # ---- DMA in ----  (small control tensors FIRST so the prologue
            # matmul chain can start while the bulk hT/adjT loads stream in)
            nc.sync.dma_start(
                hTi_sb, hTi_d[:, :].rearrange("(fc p) i -> p fc i", p=128)
            )
            nc.sync.dma_start(
                vsrc_sb, vsrc_d[:, :].rearrange("(fc p) m -> p fc m", p=128)
            )
            nc.sync.dma_start(
                wall_sb, wall_d[:, :].rearrange("(fc p) m -> p fc m", p=128)
            )
            # adjT is host-pre-tiled as [8 groups, 128 partitions, 4*NI]:
            # each group DMA is 1 MiB with 8 KiB-contiguous runs per
            # partition (large-descriptor regime, ~340 GB/s)
            adjT_r = adjT_d[:, :].rearrange("(g p) x -> g p x", p=128)
            nc.sync.dma_start(
                adjT_sb[:, 0:4, :].rearrange("p c i -> p (c i)"), adjT_r[0]
            )
            hT_r = hTf_d[:, :].rearrange("(fc p) j -> p fc j", p=128)
            nc.sync.dma_start(hT_sb[:, :, 0:512], hT_r[:, :, 0:512])
            nc.sync.dma_start(hT_sb[:, :, 512:NF], hT_r[:, :, 512:NF])
            for g in range(1, 7, 2):
                nc.sync.dma_start(
                    adjT_sb[:, 4 * g : 4 * g + 8, :].rearrange(
                        "p (g c) i -> p g (c i)", g=2
                    ),
                    adjT_r[g : g + 2].rearrange("g p x -> p g x"),
                )
            nc.sync.dma_start(
                adjT_sb[:, 28:32, :].rearrange("p c i -> p (c i)"), adjT_r[7]
            )

            nc.vector.memset(ones_row[:, :], 1.0)

            # ---- src row for this core's i-block, per head, then exp
            for h in range(NH):
                for seg in range(NSEG):
                    sr_ps = pw.tile([128, 512], f32, name=f"sr_ps_{h}_{seg}", tag="scratch")
                    for fc in range(2):
                        nc.tensor.matmul(
                            sr_ps[0:1, :],
                            lhsT=vsrc_sb[:, fc, h : h + 1],
                            rhs=hTi_sb[:, fc, ts(seg, 512)],
                            start=(fc == 0),
                            stop=(fc == 1),
                        )
                    nc.scalar.activation(
                        esrc3_row[:, h, ts(seg, 512)],
                        sr_ps[0:1, :],
                        Act.Exp,
                        scale=0.8,
                        bias=zerob[0:1, :],
                    )

            # ---- replicate e^{0.8 src} across partitions (K=1 ones matmul)
            for h in range(NH):
                for seg in range(NSEG):
                    rep_ps = pw.tile([128, 512], f32, name=f"rep_ps_{h}_{seg}", tag="scratch")
                    nc.tensor.matmul(
                        rep_ps[:, :],
                        lhsT=ones_row[:, :],
                        rhs=esrc3_row[:, h, ts(seg, 512)],
                        start=True,
                        stop=True,
                    )
                    # keep these on the DVE (baseline-tested: ACT is on the
                    # hp-chain critical path here)
                    nc.vector.tensor_copy(esrc3_rep[:, h, ts(seg, 512)], rep_ps[:, :])

            # ---- bmm accumulators: psum [65, 512] per (head, i-segment)
            acc = [
                pacc.tile([O + 1, 512], f32, name=f"acc{g}", tag=f"acc{g}")
                for g in range(NH * NSEG)
            ]

            # ---- main loop over j-chunks, with the PE/ACT producer chain
            # (h_prime -> exp scalars -> G) running PIPE chunks ahead of the
            # DVE consumer so the vector engine never stalls on it.
            PIPE = 4

            def hp_block(c):
                # h_prime (2 heads) + dst cols in one accumulated matmul
                # (chunks [0, NCI) come from the hTi tile: the host applies a
                # j-permutation putting this core's i-block columns first, so
                # those h columns are not loaded twice)
                hp_ps = pw.tile([128, 512], f32, name=f"hp_ps_{c}", tag="scratch")[:, 0:WC]
                hsrc = hTi_sb if c < NCI else hT_sb
                cc = c if c < NCI else c - NCI
                for fc in range(2):
                    nc.tensor.matmul(
                        hp_ps[:, :],
                        lhsT=hsrc[:, fc, ts(cc, 128)],
                        rhs=wall_sb[:, fc, :],
                        start=(fc == 0),
                        stop=(fc == 1),
                    )
                # one exp for all 4 dst cols (host pre-folded 0.8/0.2)
                nc.scalar.activation(
                    edst_sb[:, c, :],
                    hp_ps[:, NH * O : NH * O + 4],
                    Act.Exp,
                    scale=1.0,
                    bias=zerob[:, :],
                )
                # G = e^{0.2 dst} * [h_prime | 1] in bf16: per-head scale
                # copy plus ONE strided copy landing both heads' factors in
                # their G col-O slots (softmax denominator row)
                for h in range(NH):
                    nc.scalar.activation(
                        g_sb[:, h, c, 0:O],
                        hp_ps[:, ts(h, O)],
                        Act.Copy,
                        scale=edst_sb[:, c, 2 + h : 3 + h],
                    )
                nc.scalar.copy(
                    g_sb[:, :, c, O : O + 1],
                    edst_sb[:, c, 2:4].unsqueeze(2),
                )

            # software-pipelined main loop: every cross-engine dependency
            # gets >= 1 iteration of slack so no engine head-of-line blocks.
            #   stage H: hp_block(c+PIPE)          (PE + ACT prep)
            #   stage R: r-production for chunk c  (DVE TS or ACT relu)
            #   stage M: mask multiply for c-1     (DVE or GpSimd TT)
            #   stage A: acc matmuls for c-2       (PE; +adj term for relu)
            r_tiles, p_tiles = {}, {}

            def stage_R(c):
                r_t = sbr.tile([128, NH, NI], bf16, name=f"r_{c}", tag="R", bufs=4)
                for h in range(NH):
                    if c in RELU_CHUNKS:
                        nc.scalar.activation(
                            r_t[:, h, :],
                            esrc3_rep[:, h, :],
                            Act.Relu,
                            scale=edst_sb[:, c, h : h + 1],
                            bias=negone[:, :],
                        )
                    else:
                        nc.vector.tensor_scalar(
                            out=r_t[:, h, :],
                            in0=esrc3_rep[:, h, :],
                            scalar1=edst_sb[:, c, h : h + 1],
                            scalar2=1.0,
                            op0=Alu.mult,
                            op1=Alu.max,
                        )
                r_tiles[c] = r_t

            def stage_M(c):
                r_t = r_tiles.pop(c)
                p_t = sbr.tile([128, NH, NI], bf16, name=f"p_{c}", tag="P", bufs=4)
                for h in range(NH):
                    eng = nc.gpsimd if (c, h) in GP_TT else nc.vector
                    eng.tensor_tensor(
                        out=p_t[:, h, :],
                        in0=r_t[:, h, :],
                        in1=adjT_sb[:, c, :],
                        op=Alu.mult,
                    )
                p_tiles[c] = p_t

            def stage_A(c):
                p_t = p_tiles.pop(c)
                if c in RELU_CHUNKS:
                    for h in range(NH):
                        for seg in range(NSEG):
                            nc.tensor.matmul(
                                acc[h * NSEG + seg][:, :],
                                lhsT=g_sb[:, h, c, :],
                                rhs=adjT_sb[:, c, ts(seg, 512)],
                                start=False,
                                stop=False,
                            )
                for h in range(NH):
                    for seg in range(NSEG):
                        nc.tensor.matmul(
                            acc[h * NSEG + seg][:, :],
                            lhsT=g_sb[:, h, c, :],
                            rhs=p_t[:, h, ts(seg, 512)],
                            start=(c == 0),
                            stop=(c == NCJ - 1),
                        )

            for c in range(PIPE):
                hp_block(c)
            for it in range(NCJ + 2):
                if it + PIPE < NCJ:
                    hp_block(it + PIPE)
                if it < NCJ:
                    stage_R(it)
                if 1 <= it <= NCJ:
                    stage_M(it - 1)
                if it >= 2:
                    stage_A(it - 2)

            # ---- epilogue: copy raw accumulators (num rows 0..63, den row
            # 64) to bf16 and DMA out; host does divide/transpose/bias.
            for h in range(NH):
                for seg in range(NSEG):
                    obuf = sbo.tile([O + 1, 512], bf16, name=f"ob_{h}_{seg}", tag="ob", bufs=4)
                    nc.scalar.copy(obuf[:, :], acc[h * NSEG + seg][:, :])
                    nc.sync.dma_start(
                        out_d[h, :, ts(seg, 512)], obuf[:, :]
                    )

    nc.finalize()
    return nc


def _prep_inputs(h, adj, w, a_src, a_dst, bias):
    """Host-side sharding / layout prep (no reference math)."""
    h = np.asarray(h, dtype=np.float32)
    adj = np.asarray(adj)
    w = np.asarray(w, dtype=np.float32)
    a_src = np.asarray(a_src, dtype=np.float32)
    a_dst = np.asarray(a_dst, dtype=np.float32)
    bias = np.asarray(bias, dtype=np.float32)

    hT = np.ascontiguousarray(h.T)                       # [F, N]
    adjT = np.ascontiguousarray(adj.T).astype(ml_dtypes.bfloat16)  # [N, N] 0/1

    in_maps = []
    for c in range(NCORES):
        hb, ib = c % 2, c // 2
        heads = [2 * hb, 2 * hb + 1]
        i0 = NI * ib
        w2 = w[heads]                                    # [2, F, O]
        # parameter folding: v_src/v_dst[f] = sum_o w[f,o] * a[o]
        vsrc = np.stack(
            [w2[0] @ a_src[heads[0], :, 0], w2[1] @ a_src[heads[1], :, 0]],
            axis=1,
        )                                                # [F, 2]
        vdst = np.stack(
            [w2[0] @ a_dst[heads[0], :, 0], w2[1] @ a_dst[heads[1], :, 0]],
            axis=1,
        )                                                # [F, 2]
        wall = np.ascontiguousarray(
            np.concatenate(
                [w2.transpose(1, 0, 2).reshape(F, NH * O), 0.8 * vdst, 0.2 * vdst],
                axis=1,
            )
        )                                                # [F, WC]
        # j-permutation: this core's own i-block columns first (they ride
        # in the hTi load), remaining j's after; adjT rows follow the same
        # permutation (j is a contraction axis, so this is value-preserving)
        perm = np.r_[i0 : i0 + NI, 0:i0, i0 + NI : N]
        in_maps.append(
            {
                "hTf": np.ascontiguousarray(hT[:, perm[NI:]]).astype(
                    ml_dtypes.bfloat16
                ),
                "hTi": np.ascontiguousarray(hT[:, i0 : i0 + NI]).astype(
                    ml_dtypes.bfloat16
                ),
                "adjT": np.ascontiguousarray(
                    adjT[perm, i0 : i0 + NI]
                    .reshape(8, 4, 128, NI)
                    .transpose(0, 2, 1, 3)
                    .reshape(8 * 128, 4 * NI)
                ),
                "wall": wall.astype(ml_dtypes.bfloat16),
                "vsrc": np.ascontiguousarray(vsrc).astype(ml_dtypes.bfloat16),
            }
        )
    return in_maps


def kernel(h, adj, w, a_src, a_dst, bias):
    from concourse.bass_utils import run_bass_kernel_spmd

    if "nc" not in _CACHE:
        _CACHE["nc"] = _build()
    nc = _CACHE["nc"]

    in_maps = _prep_inputs(h, adj, w, a_src, a_dst, bias)
    res = run_bass_kernel_spmd(nc, in_maps, list(range(NCORES))).results

    out = np.empty((N, NHEAD, O), dtype=np.float32)
    for c in range(NCORES):
        hb, ib = c % 2, c // 2
        arr = np.asarray(res[c]["out"], dtype=np.float32)  # [NH, O+1, NI]
        for hh in range(NH):
            num = arr[hh, 0:O, :]                          # [O, NI]
            den = arr[hh, O, :]                            # [NI]
            out[NI * ib : NI * (ib + 1), 2 * hb + hh, :] = (num / den).T
    out += np.asarray(bias, dtype=np.float32).reshape(1, 1, O)
    return out


# revision 28
# speedup vs baseline: 1.0726x; 1.0726x over previous
"""Multi-head graph attention (GAT) Trainium2 kernel, 8-core SPMD.

Problem: h[4096,256], adj[4096,4096] bool, w[4,256,64], a_src/a_dst[4,64,1],
bias[64] -> out[4096,4,64]:
    h_prime = h @ w[k]                       per head
    s[i,j]  = src[i] + dst[j]                (rank-1!)
    scores  = leaky_relu(s, 0.2), masked by adj, softmax over j
    out     = attn @ h_prime + bias

Sharding: 8 cores = 2 head-groups x 4 row-blocks. Core c computes heads
[2*(c%2), 2*(c%2)+1] for output rows [1024*(c//2), 1024*(c//2)+1024).

Key algebra (all on-device, per head):
    exp(leaky(s)) = e^{0.2 s} * max(e^{0.8 s}, 1);  e^{0.8 s} = esrc_i*edst_j
The per-column factor e^{0.2 src_i} cancels in the softmax normalization,
so the unnormalized weights reduce to
    P'[j,i] = e^{0.2 dst_j} * adj[i,j] * max(e^{0.8 src_i} * e^{0.8 dst_j}, 1)
The e^{0.2 dst_j} row-factor rides the stationary G operand of the final
bmm (col O of G holds the factor itself -> softmax denominator row).

v2.1 structure (DVE is the wall; measured: GpSimd software tensor ops are
~10x the cost model -- useless for bulk work; stride-0 broadcast TT also
falls off the DVE fast path):
  - classic chunks: r = max(esrc*edst, 1) via DVE tensor_scalar, then
    p = r * adjT via DVE tensor_tensor, per head.
  - RELU_CHUNKS use max(x,1)*adj = relu(x-1)*adj + adj: the relu runs on
    the otherwise-slack ACT engine, and the "+adj" term is two extra PE
    matmuls accumulating bmm(G, adjT-chunk) into the same psum acc --
    issued inside hp_block, ahead of the DVE, where the PE is idle.
  - host folds v = w @ a (parameter folding) and pre-scales the dst
    columns by 0.8/0.2 so ONE exp covers all 4 exp scalars per chunk.
  - epilogue: accumulators [65,512] are copied (ACT) to bf16 and DMA'd
    raw; the num/den division, transpose, and bias happen on HOST during
    unsharding (pure layout/normalization of the device-computed sums).
"""

import sys

sys.path.insert(0, "/opt/trn_rl_repo")

import numpy as np
import ml_dtypes

N = 4096          # nodes
F = 256           # f_in
O = 64            # f_out
NHEAD = 4
NCORES = 8
NH = 2            # heads per core
NI = 1024         # output rows per core
NCJ = N // 128    # 32 j-chunks
NSEG = NI // 512  # 2 segments of 512 in the i (free) dim
NF = N - NI       # 3072 j-columns outside this core's i-block
NCI = NI // 128   # 8 j-chunks whose h columns come from the hTi tile
WC = NH * O + 4 * NH  # wall: [hp h0|hp h1| 0.8v | 0.2v | -0.8v | 1.0v]

# chunks using the relu decomposition: max(x,1)*adj = relu(x-1)*adj + adj.
# The relu runs on ACT (off the DVE critical path) and the "+adj" term is
# two extra PE matmuls accumulating bmm(G, adjT_chunk) into the same psum.
RELU_CHUNKS = frozenset({1, 5, 9})
# chunks using the fused scalar_tensor_tensor form:
#   p = (esrc_rep MAX e^{-0.8 dst_j}) MULT adjT,  G carries e^{1.0 dst_j}
# (identical per-element products as the classic form; the per-j factor
# moves wholly onto G).  One DVE op instead of TS+TT -- HW probe: the
# cost model says STT has no fast modes (1127ns, a wash vs 1171ns for
# the pair), but it was wrong about GpSimd in both directions.
STT_CHUNKS = frozenset(range(12, NCJ))

_CACHE = {}


def _build():
    import concourse.bass as bass
    import concourse.bacc as bacc
    import concourse.mybir as mybir
    import concourse.tile as tile
    from concourse.bass import ts

    f32 = mybir.dt.float32
    bf16 = mybir.dt.bfloat16
    Alu = mybir.AluOpType
    Act = mybir.ActivationFunctionType

    nc = bacc.Bacc()
    hTf_d = nc.declare_dram_parameter("hTf", [F, NF], bf16, isOutput=False)
    hTi_d = nc.declare_dram_parameter("hTi", [F, NI], bf16, isOutput=False)
    adjT_d = nc.declare_dram_parameter("adjT", [8 * 128, 4 * NI], bf16, isOutput=False)
    # wall = [w (head-major) | 0.8*v_dst | 0.2*v_dst], v = w @ a folded on
    # host, exponent scales pre-folded so ONE exp op covers all 4 columns
    wall_d = nc.declare_dram_parameter("wall", [F, WC], bf16, isOutput=False)
    vsrc_d = nc.declare_dram_parameter("vsrc", [F, NH], bf16, isOutput=False)
    # raw accumulator output: [head, O+1 rows (num | den), i] in bf16
    out_d = nc.declare_dram_parameter("out", [NH, O + 1, NI], bf16, isOutput=True)

    with tile.TileContext(nc) as tc:
        with (
            tc.tile_pool(name="sb", bufs=1) as sb,
            tc.tile_pool(name="sbr", bufs=2) as sbr,
            tc.tile_pool(name="sbo", bufs=2) as sbo,
            tc.tile_pool(name="pw", bufs=4, space="PSUM") as pw,
            tc.tile_pool(name="pacc", bufs=1, space="PSUM") as pacc,
        ):
            # ---- static SBUF tensors ----
            hT_sb = sb.tile([128, 2, NF], bf16, name="hT_sb")
            hTi_sb = sb.tile([128, 2, NI], bf16, name="hTi_sb")
            adjT_sb = sb.tile([128, NCJ, NI], bf16, name="adjT_sb")
            wall_sb = sb.tile([128, 2, WC], bf16, name="wall_sb")
            vsrc_sb = sb.tile([128, 2, NH], bf16, name="vsrc_sb")
            ones_row = sb.tile([1, 128], bf16, name="ones_row")
            esrc3_rep = sb.tile([128, NH, NI], bf16, name="esrc3_rep")
            esrc3_row = sb.tile([1, NH, NI], bf16, name="esrc3_row")
            g_sb = sb.tile([128, NH, NCJ, O + 1], bf16, name="g_sb")
            # cols 0-1: e^{0.8 dst} (TS/relu scalar), 2-3: e^{0.2 dst}
            # (classic G), 4-5: e^{-0.8 dst} (STT scalar), 6-7: e^{dst}
            # (STT G) -- all from ONE exp (scales folded into wall)
            edst_sb = sb.tile([128, NCJ, 8], f32, name="edst_sb")
            zerob = sb.tile([128, 1], f32, name="zerob")
            nc.vector.memset(zerob[:, :], 0.0)
            negone = sb.tile([128, 1], f32, name="negone")
            nc.vector.memset(negone[:, :], -1.0)
            # NOTE: do NOT add an exp-table warm-up op here -- tested three
            # times in the baseline, always slower.

            # ---- DMA in ----  (small control tensors FIRST so the prologue
            # matmul chain can start while the bulk hT/adjT loads stream in)
            hTi_r = hTi_d[:, :].rearrange("(fc p) i -> p fc i", p=128)
            nc.sync.dma_start(hTi_sb[:, :, 0:512], hTi_r[:, :, 0:512])
            nc.sync.dma_start(hTi_sb[:, :, 512:NI], hTi_r[:, :, 512:NI])
            nc.sync.dma_start(
                vsrc_sb, vsrc_d[:, :].rearrange("(fc p) m -> p fc m", p=128)
            )
            nc.sync.dma_start(
                wall_sb, wall_d[:, :].rearrange("(fc p) m -> p fc m", p=128)
            )
            # adjT is host-pre-tiled as [8 groups, 128 partitions, 4*NI]:
            # each group DMA is 1 MiB with 8 KiB-contiguous runs per
            # partition (large-descriptor regime, ~340 GB/s)
            adjT_r = adjT_d[:, :].rearrange("(g p) x -> g p x", p=128)
            nc.sync.dma_start(
                adjT_sb[:, 0:4, :].rearrange("p c i -> p (c i)"), adjT_r[0]
            )
            hT_r = hTf_d[:, :].rearrange("(fc p) j -> p fc j", p=128)
            nc.sync.dma_start(hT_sb[:, :, 0:512], hT_r[:, :, 0:512])
            nc.sync.dma_start(hT_sb[:, :, 512:NF], hT_r[:, :, 512:NF])
            for g in range(1, 7, 2):
                nc.sync.dma_start(
                    adjT_sb[:, 4 * g : 4 * g + 8, :].rearrange(
                        "p (g c) i -> p g (c i)", g=2
                    ),
                    adjT_r[g : g + 2].rearrange("g p x -> p g x"),
                )
            nc.sync.dma_start(
                adjT_sb[:, 28:32, :].rearrange("p c i -> p (c i)"), adjT_r[7]
            )

            nc.vector.memset(ones_row[:, :], 1.0)

            # ---- src row for this core's i-block, per head, then exp
            for h in range(NH):
                for seg in range(NSEG):
                    sr_ps = pw.tile([128, 512], f32, name=f"sr_ps_{h}_{seg}", tag="scratch")
                    for fc in range(2):
                        nc.tensor.matmul(
                            sr_ps[0:1, :],
                            lhsT=vsrc_sb[:, fc, h : h + 1],
                            rhs=hTi_sb[:, fc, ts(seg, 512)],
                            start=(fc == 0),
                            stop=(fc == 1),
                        )
                    nc.scalar.activation(
                        esrc3_row[:, h, ts(seg, 512)],
                        sr_ps[0:1, :],
                        Act.Exp,
                        scale=0.8,
                        bias=zerob[0:1, :],
                    )

            # ---- replicate e^{0.8 src} across partitions (K=1 ones matmul)
            for h in range(NH):
                for seg in range(NSEG):
                    rep_ps = pw.tile([128, 512], f32, name=f"rep_ps_{h}_{seg}", tag="scratch")
                    nc.tensor.matmul(
                        rep_ps[:, :],
                        lhsT=ones_row[:, :],
                        rhs=esrc3_row[:, h, ts(seg, 512)],
                        start=True,
                        stop=True,
                    )
                    # keep these on the DVE (baseline-tested: ACT is on the
                    # hp-chain critical path here)
                    nc.vector.tensor_copy(esrc3_rep[:, h, ts(seg, 512)], rep_ps[:, :])

            # ---- bmm accumulators: psum [65, 512] per (head, i-segment)
            acc = [
                pacc.tile([O + 1, 512], f32, name=f"acc{g}", tag=f"acc{g}")
                for g in range(NH * NSEG)
            ]

            # ---- main loop over j-chunks, with the PE/ACT producer chain
            # (h_prime -> exp scalars -> G) running PIPE chunks ahead of the
            # DVE consumer so the vector engine never stalls on it.
            PIPE = 4

            def hp_block(c):
                # h_prime (2 heads) + dst cols in one accumulated matmul
                # (chunks [0, NCI) come from the hTi tile: the host applies a
                # j-permutation putting this core's i-block columns first, so
                # those h columns are not loaded twice)
                hp_ps = pw.tile([128, 512], f32, name=f"hp_ps_{c}", tag="scratch")[:, 0:WC]
                hsrc = hTi_sb if c < NCI else hT_sb
                cc = c if c < NCI else c - NCI
                for fc in range(2):
                    nc.tensor.matmul(
                        hp_ps[:, :],
                        lhsT=hsrc[:, fc, ts(cc, 128)],
                        rhs=wall_sb[:, fc, :],
                        start=(fc == 0),
                        stop=(fc == 1),
                    )
                # one exp for all 8 dst cols (scales pre-folded into wall)
                nc.scalar.activation(
                    edst_sb[:, c, :],
                    hp_ps[:, NH * O : NH * O + 8],
                    Act.Exp,
                    scale=1.0,
                    bias=zerob[:, :],
                )
                # G = gf * [h_prime | 1] in bf16, gf = e^{0.2 dst} (classic)
                # or e^{dst} (STT form): per-head scale copy plus ONE strided
                # copy landing both heads' factors in their G col-O slots
                # (softmax denominator row)
                gc = 6 if c in STT_CHUNKS else 2
                for h in range(NH):
                    nc.scalar.activation(
                        g_sb[:, h, c, 0:O],
                        hp_ps[:, ts(h, O)],
                        Act.Copy,
                        scale=edst_sb[:, c, gc + h : gc + h + 1],
                    )
                nc.scalar.copy(
                    g_sb[:, :, c, O : O + 1],
                    edst_sb[:, c, gc : gc + 2].unsqueeze(2),
                )

            # software-pipelined main loop: every cross-engine dependency
            # gets >= 1 iteration of slack so no engine head-of-line blocks.
            #   stage H: hp_block(c+PIPE)          (PE + ACT prep)
            #   stage R: r-production for chunk c  (DVE TS or ACT relu)
            #   stage M: mask multiply for c-1     (DVE or GpSimd TT)
            #   stage A: acc matmuls for c-2       (PE; +adj term for relu)
            r_tiles, p_tiles = {}, {}

            def stage_R(c):
                if c in STT_CHUNKS:
                    return
                r_t = sbr.tile([128, NH, NI], bf16, name=f"r_{c}", tag="R", bufs=4)
                for h in range(NH):
                    if c in RELU_CHUNKS:
                        nc.scalar.activation(
                            r_t[:, h, :],
                            esrc3_rep[:, h, :],
                            Act.Relu,
                            scale=edst_sb[:, c, h : h + 1],
                            bias=negone[:, :],
                        )
                    else:
                        nc.vector.tensor_scalar(
                            out=r_t[:, h, :],
                            in0=esrc3_rep[:, h, :],
                            scalar1=edst_sb[:, c, h : h + 1],
                            scalar2=1.0,
                            op0=Alu.mult,
                            op1=Alu.max,
                        )
                r_tiles[c] = r_t

            def stage_M(c):
                p_t = sbr.tile([128, NH, NI], bf16, name=f"p_{c}", tag="P", bufs=4)
                p_tiles[c] = p_t
                if c in STT_CHUNKS:
                    for h in range(NH):
                        nc.vector.scalar_tensor_tensor(
                            out=p_t[:, h, :],
                            in0=esrc3_rep[:, h, :],
                            scalar=edst_sb[:, c, 4 + h : 5 + h],
                            in1=adjT_sb[:, c, :],
                            op0=Alu.max,
                            op1=Alu.mult,
                        )
                    return
                r_t = r_tiles.pop(c)
                for h in range(NH):
                    nc.vector.tensor_tensor(
                        out=p_t[:, h, :],
                        in0=r_t[:, h, :],
                        in1=adjT_sb[:, c, :],
                        op=Alu.mult,
                    )

            def stage_A(c):
                p_t = p_tiles.pop(c)
                if c in RELU_CHUNKS:
                    for h in range(NH):
                        for seg in range(NSEG):
                            nc.tensor.matmul(
                                acc[h * NSEG + seg][:, :],
                                lhsT=g_sb[:, h, c, :],
                                rhs=adjT_sb[:, c, ts(seg, 512)],
                                start=False,
                                stop=False,
                            )
                for h in range(NH):
                    for seg in range(NSEG):
                        nc.tensor.matmul(
                            acc[h * NSEG + seg][:, :],
                            lhsT=g_sb[:, h, c, :],
                            rhs=p_t[:, h, ts(seg, 512)],
                            start=(c == 0),
                            stop=(c == NCJ - 1),
                        )

            for c in range(PIPE):
                hp_block(c)
            for it in range(NCJ + 2):
                if it + PIPE < NCJ:
                    hp_block(it + PIPE)
                if it < NCJ:
                    stage_R(it)
                if 1 <= it <= NCJ:
                    stage_M(it - 1)
                if it >= 2:
                    stage_A(it - 2)

            # ---- epilogue: copy raw accumulators (num rows 0..63, den row
            # 64) to bf16 and DMA out; host does divide/transpose/bias.
            for h in range(NH):
                for seg in range(NSEG):
                    obuf = sbo.tile([O + 1, 512], bf16, name=f"ob_{h}_{seg}", tag="ob", bufs=4)
                    nc.scalar.copy(obuf[:, :], acc[h * NSEG + seg][:, :])
                    nc.sync.dma_start(
                        out_d[h, :, ts(seg, 512)], obuf[:, :]
                    )

    nc.finalize()
    return nc


def _prep_inputs(h, adj, w, a_src, a_dst, bias):
    """Host-side sharding / layout prep (no reference math)."""
    h = np.asarray(h, dtype=np.float32)
    adj = np.asarray(adj)
    w = np.asarray(w, dtype=np.float32)
    a_src = np.asarray(a_src, dtype=np.float32)
    a_dst = np.asarray(a_dst, dtype=np.float32)
    bias = np.asarray(bias, dtype=np.float32)

    hT = np.ascontiguousarray(h.T)                       # [F, N]
    adjT = np.ascontiguousarray(adj.T).astype(ml_dtypes.bfloat16)  # [N, N] 0/1

    in_maps = []
    for c in range(NCORES):
        hb, ib = c % 2, c // 2
        heads = [2 * hb, 2 * hb + 1]
        i0 = NI * ib
        w2 = w[heads]                                    # [2, F, O]
        # parameter folding: v_src/v_dst[f] = sum_o w[f,o] * a[o]
        vsrc = np.stack(
            [w2[0] @ a_src[heads[0], :, 0], w2[1] @ a_src[heads[1], :, 0]],
            axis=1,
        )                                                # [F, 2]
        vdst = np.stack(
            [w2[0] @ a_dst[heads[0], :, 0], w2[1] @ a_dst[heads[1], :, 0]],
            axis=1,
        )                                                # [F, 2]
        wall = np.ascontiguousarray(
            np.concatenate(
                [w2.transpose(1, 0, 2).reshape(F, NH * O),
                 0.8 * vdst, 0.2 * vdst, -0.8 * vdst, vdst],
                axis=1,
            )
        )                                                # [F, WC]
        # j-permutation: this core's own i-block columns first (they ride
        # in the hTi load), remaining j's after; adjT rows follow the same
        # permutation (j is a contraction axis, so this is value-preserving)
        perm = np.r_[i0 : i0 + NI, 0:i0, i0 + NI : N]
        in_maps.append(
            {
                "hTf": np.ascontiguousarray(hT[:, perm[NI:]]).astype(
                    ml_dtypes.bfloat16
                ),
                "hTi": np.ascontiguousarray(hT[:, i0 : i0 + NI]).astype(
                    ml_dtypes.bfloat16
                ),
                "adjT": np.ascontiguousarray(
                    adjT[perm, i0 : i0 + NI]
                    .reshape(8, 4, 128, NI)
                    .transpose(0, 2, 1, 3)
                    .reshape(8 * 128, 4 * NI)
                ),
                "wall": wall.astype(ml_dtypes.bfloat16),
                "vsrc": np.ascontiguousarray(vsrc).astype(ml_dtypes.bfloat16),
            }
        )
    return in_maps


def kernel(h, adj, w, a_src, a_dst, bias):
    from concourse.bass_utils import run_bass_kernel_spmd

    if "nc" not in _CACHE:
        _CACHE["nc"] = _build()
    nc = _CACHE["nc"]

    in_maps = _prep_inputs(h, adj, w, a_src, a_dst, bias)
    res = run_bass_kernel_spmd(nc, in_maps, list(range(NCORES))).results

    out = np.empty((N, NHEAD, O), dtype=np.float32)
    for c in range(NCORES):
        hb, ib = c % 2, c // 2
        arr = np.asarray(res[c]["out"], dtype=np.float32)  # [NH, O+1, NI]
        for hh in range(NH):
            num = arr[hh, 0:O, :]                          # [O, NI]
            den = arr[hh, O, :]                            # [NI]
            out[NI * ib : NI * (ib + 1), 2 * hb + hh, :] = (num / den).T
    out += np.asarray(bias, dtype=np.float32).reshape(1, 1, O)
    return out


# revision 30
# speedup vs baseline: 1.2264x; 1.1434x over previous
"""Multi-head graph attention (GAT) Trainium2 kernel, 8-core SPMD.

Problem: h[4096,256], adj[4096,4096] bool, w[4,256,64], a_src/a_dst[4,64,1],
bias[64] -> out[4096,4,64]:
    h_prime = h @ w[k]                       per head
    s[i,j]  = src[i] + dst[j]                (rank-1!)
    scores  = leaky_relu(s, 0.2), masked by adj, softmax over j
    out     = attn @ h_prime + bias

Sharding: 8 cores = 2 head-groups x 4 row-blocks. Core c computes heads
[2*(c%2), 2*(c%2)+1] for output rows [1024*(c//2), 1024*(c//2)+1024).

Key algebra (all on-device, per head):
    exp(leaky(s)) = e^{0.2 s} * max(e^{0.8 s}, 1);  e^{0.8 s} = esrc_i*edst_j
The per-column factor e^{0.2 src_i} cancels in the softmax normalization,
so the unnormalized weights reduce to
    P'[j,i] = e^{0.2 dst_j} * adj[i,j] * max(e^{0.8 src_i} * e^{0.8 dst_j}, 1)
The e^{0.2 dst_j} row-factor rides the stationary G operand of the final
bmm (col O of G holds the factor itself -> softmax denominator row).

v2.1 structure (DVE is the wall; measured: GpSimd software tensor ops are
~10x the cost model -- useless for bulk work; stride-0 broadcast TT also
falls off the DVE fast path):
  - classic chunks: r = max(esrc*edst, 1) via DVE tensor_scalar, then
    p = r * adjT via DVE tensor_tensor, per head.
  - RELU_CHUNKS use max(x,1)*adj = relu(x-1)*adj + adj: the relu runs on
    the otherwise-slack ACT engine, and the "+adj" term is two extra PE
    matmuls accumulating bmm(G, adjT-chunk) into the same psum acc --
    issued inside hp_block, ahead of the DVE, where the PE is idle.
  - host folds v = w @ a (parameter folding) and pre-scales the dst
    columns by 0.8/0.2 so ONE exp covers all 4 exp scalars per chunk.
  - epilogue: accumulators [65,512] are copied (ACT) to bf16 and DMA'd
    raw; the num/den division, transpose, and bias happen on HOST during
    unsharding (pure layout/normalization of the device-computed sums).
"""

import sys

sys.path.insert(0, "/opt/trn_rl_repo")

import numpy as np
import ml_dtypes

N = 4096          # nodes
F = 256           # f_in
O = 64            # f_out
NHEAD = 4
NCORES = 8
NH = 2            # heads per core
NI = 1024         # output rows per core
NCJ = N // 128    # 32 j-chunks
NSEG = NI // 512  # 2 segments of 512 in the i (free) dim
NF = N - NI       # 3072 j-columns outside this core's i-block
NCI = NI // 128   # 8 j-chunks whose h columns come from the hTi tile
WC = NH * O + 4 * NH  # wall: [hp h0|hp h1| 0.8v | 0.2v | -0.8v | 1.0v]

# chunks using the relu decomposition: max(x,1)*adj = relu(x-1)*adj + adj.
# The relu runs on ACT (off the DVE critical path) and the "+adj" term is
# two extra PE matmuls accumulating bmm(G, adjT_chunk) into the same psum.
# Relu chunks: max(x,1)*adj = relu(x-1)*adj + adj.  The relu runs on the
# slack ACT engine (r~ produced 2 chunks ahead), the mask TT runs 1 chunk
# ahead, and the "+adj" term is a DMA-engine accumulate (accum_op=add)
# over the p tile -- zero PE/DVE cost.  Chunks sit late so the adj-add
# DMAs don't contend with the input adjT stream.
# (Measured dead ends: GpSimd tensor ops 3-10x cost model + slow the DVE
# ~30% via SBUF contention; DVE scalar_tensor_tensor is 1x = 1280ns, no
# win over TS+TT; stride-0 broadcast TT falls off the fast path; DMA
# accum supports ONLY add.)
RELU_CHUNKS = frozenset({9, 11, 13, 15, 17, 19, 21, 23, 25, 27})
STT_CHUNKS = frozenset()

_CACHE = {}


def _build():
    import concourse.bass as bass
    import concourse.bacc as bacc
    import concourse.mybir as mybir
    import concourse.tile as tile
    from concourse.bass import ts

    f32 = mybir.dt.float32
    bf16 = mybir.dt.bfloat16
    Alu = mybir.AluOpType
    Act = mybir.ActivationFunctionType

    nc = bacc.Bacc()
    hTf_d = nc.declare_dram_parameter("hTf", [F, NF], bf16, isOutput=False)
    hTi_d = nc.declare_dram_parameter("hTi", [F, NI], bf16, isOutput=False)
    adjT_d = nc.declare_dram_parameter("adjT", [8 * 128, 4 * NI], bf16, isOutput=False)
    # wall = [w (head-major) | 0.8*v_dst | 0.2*v_dst], v = w @ a folded on
    # host, exponent scales pre-folded so ONE exp op covers all 4 columns
    wall_d = nc.declare_dram_parameter("wall", [F, WC], bf16, isOutput=False)
    vsrc_d = nc.declare_dram_parameter("vsrc", [F, NH], bf16, isOutput=False)
    # raw accumulator output: [head, O+1 rows (num | den), i] in bf16
    out_d = nc.declare_dram_parameter("out", [NH, O + 1, NI], bf16, isOutput=True)

    with tile.TileContext(nc) as tc:
        with (
            tc.tile_pool(name="sb", bufs=1) as sb,
            tc.tile_pool(name="sbr", bufs=2) as sbr,
            tc.tile_pool(name="sbo", bufs=2) as sbo,
            tc.tile_pool(name="pw", bufs=4, space="PSUM") as pw,
            tc.tile_pool(name="pacc", bufs=1, space="PSUM") as pacc,
        ):
            # ---- static SBUF tensors ----
            hT_sb = sb.tile([128, 2, NF], bf16, name="hT_sb")
            hTi_sb = sb.tile([128, 2, NI], bf16, name="hTi_sb")
            adjT_sb = sb.tile([128, NCJ, NI], bf16, name="adjT_sb")
            wall_sb = sb.tile([128, 2, WC], bf16, name="wall_sb")
            vsrc_sb = sb.tile([128, 2, NH], bf16, name="vsrc_sb")
            ones_row = sb.tile([1, 128], bf16, name="ones_row")
            esrc3_rep = sb.tile([128, NH, NI], bf16, name="esrc3_rep")
            esrc3_row = sb.tile([1, NH, NI], bf16, name="esrc3_row")
            g_sb = sb.tile([128, NH, NCJ, O + 1], bf16, name="g_sb")
            # cols 0-1: e^{0.8 dst} (TS/relu scalar), 2-3: e^{0.2 dst}
            # (classic G), 4-5: e^{-0.8 dst} (STT scalar), 6-7: e^{dst}
            # (STT G) -- all from ONE exp (scales folded into wall)
            edst_sb = sb.tile([128, NCJ, 8], f32, name="edst_sb")
            zerob = sb.tile([128, 1], f32, name="zerob")
            nc.vector.memset(zerob[:, :], 0.0)
            negone = sb.tile([128, 1], f32, name="negone")
            nc.vector.memset(negone[:, :], -1.0)
            # NOTE: do NOT add an exp-table warm-up op here -- tested three
            # times in the baseline, always slower.

            # ---- DMA in ----  (small control tensors FIRST so the prologue
            # matmul chain can start while the bulk hT/adjT loads stream in)
            hTi_r = hTi_d[:, :].rearrange("(fc p) i -> p fc i", p=128)
            nc.sync.dma_start(hTi_sb[:, :, 0:512], hTi_r[:, :, 0:512])
            nc.sync.dma_start(hTi_sb[:, :, 512:NI], hTi_r[:, :, 512:NI])
            nc.sync.dma_start(
                vsrc_sb, vsrc_d[:, :].rearrange("(fc p) m -> p fc m", p=128)
            )
            nc.sync.dma_start(
                wall_sb, wall_d[:, :].rearrange("(fc p) m -> p fc m", p=128)
            )
            # adjT is host-pre-tiled as [8 groups, 128 partitions, 4*NI]:
            # each group DMA is 1 MiB with 8 KiB-contiguous runs per
            # partition (large-descriptor regime, ~340 GB/s)
            adjT_r = adjT_d[:, :].rearrange("(g p) x -> g p x", p=128)
            nc.sync.dma_start(
                adjT_sb[:, 0:4, :].rearrange("p c i -> p (c i)"), adjT_r[0]
            )
            hT_r = hTf_d[:, :].rearrange("(fc p) j -> p fc j", p=128)
            nc.sync.dma_start(hT_sb[:, :, 0:512], hT_r[:, :, 0:512])
            nc.sync.dma_start(hT_sb[:, :, 512:NF], hT_r[:, :, 512:NF])
            for g in range(1, 7, 2):
                nc.sync.dma_start(
                    adjT_sb[:, 4 * g : 4 * g + 8, :].rearrange(
                        "p (g c) i -> p g (c i)", g=2
                    ),
                    adjT_r[g : g + 2].rearrange("g p x -> p g x"),
                )
            nc.sync.dma_start(
                adjT_sb[:, 28:32, :].rearrange("p c i -> p (c i)"), adjT_r[7]
            )

            nc.vector.memset(ones_row[:, :], 1.0)

            # ---- src row for this core's i-block, per head, then exp
            for h in range(NH):
                for seg in range(NSEG):
                    sr_ps = pw.tile([128, 512], f32, name=f"sr_ps_{h}_{seg}", tag="scratch")
                    for fc in range(2):
                        nc.tensor.matmul(
                            sr_ps[0:1, :],
                            lhsT=vsrc_sb[:, fc, h : h + 1],
                            rhs=hTi_sb[:, fc, ts(seg, 512)],
                            start=(fc == 0),
                            stop=(fc == 1),
                        )
                    nc.scalar.activation(
                        esrc3_row[:, h, ts(seg, 512)],
                        sr_ps[0:1, :],
                        Act.Exp,
                        scale=0.8,
                        bias=zerob[0:1, :],
                    )

            # ---- replicate e^{0.8 src} across partitions (K=1 ones matmul)
            for h in range(NH):
                for seg in range(NSEG):
                    rep_ps = pw.tile([128, 512], f32, name=f"rep_ps_{h}_{seg}", tag="scratch")
                    nc.tensor.matmul(
                        rep_ps[:, :],
                        lhsT=ones_row[:, :],
                        rhs=esrc3_row[:, h, ts(seg, 512)],
                        start=True,
                        stop=True,
                    )
                    # keep these on the DVE (baseline-tested: ACT is on the
                    # hp-chain critical path here)
                    nc.vector.tensor_copy(esrc3_rep[:, h, ts(seg, 512)], rep_ps[:, :])

            # ---- bmm accumulators: psum [65, 512] per (head, i-segment)
            acc = [
                pacc.tile([O + 1, 512], f32, name=f"acc{g}", tag=f"acc{g}")
                for g in range(NH * NSEG)
            ]

            # ---- main loop over j-chunks, with the PE/ACT producer chain
            # (h_prime -> exp scalars -> G) running PIPE chunks ahead of the
            # DVE consumer so the vector engine never stalls on it.
            PIPE = 4

            def hp_block(c):
                # h_prime (2 heads) + dst cols in one accumulated matmul
                # (chunks [0, NCI) come from the hTi tile: the host applies a
                # j-permutation putting this core's i-block columns first, so
                # those h columns are not loaded twice)
                hp_ps = pw.tile([128, 512], f32, name=f"hp_ps_{c}", tag="scratch")[:, 0:WC]
                hsrc = hTi_sb if c < NCI else hT_sb
                cc = c if c < NCI else c - NCI
                for fc in range(2):
                    nc.tensor.matmul(
                        hp_ps[:, :],
                        lhsT=hsrc[:, fc, ts(cc, 128)],
                        rhs=wall_sb[:, fc, :],
                        start=(fc == 0),
                        stop=(fc == 1),
                    )
                # one exp for all 8 dst cols (scales pre-folded into wall)
                nc.scalar.activation(
                    edst_sb[:, c, :],
                    hp_ps[:, NH * O : NH * O + 8],
                    Act.Exp,
                    scale=1.0,
                    bias=zerob[:, :],
                )
                # G = gf * [h_prime | 1] in bf16, gf = e^{0.2 dst} (classic)
                # or e^{dst} (STT form): per-head scale copy plus ONE strided
                # copy landing both heads' factors in their G col-O slots
                # (softmax denominator row)
                gc = 6 if c in STT_CHUNKS else 2
                for h in range(NH):
                    nc.scalar.activation(
                        g_sb[:, h, c, 0:O],
                        hp_ps[:, ts(h, O)],
                        Act.Copy,
                        scale=edst_sb[:, c, gc + h : gc + h + 1],
                    )
                nc.scalar.copy(
                    g_sb[:, :, c, O : O + 1],
                    edst_sb[:, c, gc : gc + 2].unsqueeze(2),
                )

            # main loop, v2.1-style tight schedule.  Relu chunks run their
            # ACT r~ two iterations early and their TT + adj-add one early,
            # so the DMA-add latency (~2us) is hidden; classic chunks stay
            # tight (TS -> TT -> matmuls same iteration).
            r_tiles, p_tiles = {}, {}

            def relu_R(c):
                r_t = sbr.tile([128, NH, NI], bf16, name=f"r_{c}", tag="R", bufs=4)
                for h in range(NH):
                    nc.scalar.activation(
                        r_t[:, h, :],
                        esrc3_rep[:, h, :],
                        Act.Relu,
                        scale=edst_sb[:, c, h : h + 1],
                        bias=negone[:, :],
                    )
                r_tiles[c] = r_t

            def mask_M(c):
                # p = r * adjT; relu chunks then accumulate +adjT on the
                # DMA engines (exact: (relu(x-1))*adj + adj = max(x,1)*adj)
                r_t = r_tiles.pop(c)
                p_t = sbr.tile([128, NH, NI], bf16, name=f"p_{c}", tag="P", bufs=4)
                p_tiles[c] = p_t
                for h in range(NH):
                    nc.vector.tensor_tensor(
                        out=p_t[:, h, :],
                        in0=r_t[:, h, :],
                        in1=adjT_sb[:, c, :],
                        op=Alu.mult,
                    )
                    if c in RELU_CHUNKS:
                        nc.gpsimd.dma_start(
                            p_t[:, h, :],
                            adjT_sb[:, c, :],
                            accum_op=Alu.add,
                        )

            for c in range(PIPE):
                hp_block(c)
            for c in range(NCJ):
                if c + PIPE < NCJ:
                    hp_block(c + PIPE)
                if c + 2 < NCJ and (c + 2) in RELU_CHUNKS:
                    relu_R(c + 2)
                if c + 1 < NCJ and (c + 1) in RELU_CHUNKS:
                    mask_M(c + 1)
                if c not in RELU_CHUNKS:
                    r_t = sbr.tile([128, NH, NI], bf16, name=f"r_{c}", tag="R", bufs=4)
                    for h in range(NH):
                        nc.vector.tensor_scalar(
                            out=r_t[:, h, :],
                            in0=esrc3_rep[:, h, :],
                            scalar1=edst_sb[:, c, h : h + 1],
                            scalar2=1.0,
                            op0=Alu.mult,
                            op1=Alu.max,
                        )
                    r_tiles[c] = r_t
                    mask_M(c)
                p_t = p_tiles.pop(c)
                for h in range(NH):
                    for seg in range(NSEG):
                        nc.tensor.matmul(
                            acc[h * NSEG + seg][:, :],
                            lhsT=g_sb[:, h, c, :],
                            rhs=p_t[:, h, ts(seg, 512)],
                            start=(c == 0),
                            stop=(c == NCJ - 1),
                        )

            # ---- epilogue: copy raw accumulators (num rows 0..63, den row
            # 64) to bf16 and DMA out; host does divide/transpose/bias.
            for h in range(NH):
                for seg in range(NSEG):
                    obuf = sbo.tile([O + 1, 512], bf16, name=f"ob_{h}_{seg}", tag="ob", bufs=4)
                    nc.scalar.copy(obuf[:, :], acc[h * NSEG + seg][:, :])
                    nc.sync.dma_start(
                        out_d[h, :, ts(seg, 512)], obuf[:, :]
                    )

    nc.finalize()
    return nc


def _prep_inputs(h, adj, w, a_src, a_dst, bias):
    """Host-side sharding / layout prep (no reference math)."""
    h = np.asarray(h, dtype=np.float32)
    adj = np.asarray(adj)
    w = np.asarray(w, dtype=np.float32)
    a_src = np.asarray(a_src, dtype=np.float32)
    a_dst = np.asarray(a_dst, dtype=np.float32)
    bias = np.asarray(bias, dtype=np.float32)

    hT = np.ascontiguousarray(h.T)                       # [F, N]
    adjT = np.ascontiguousarray(adj.T).astype(ml_dtypes.bfloat16)  # [N, N] 0/1

    in_maps = []
    for c in range(NCORES):
        hb, ib = c % 2, c // 2
        heads = [2 * hb, 2 * hb + 1]
        i0 = NI * ib
        w2 = w[heads]                                    # [2, F, O]
        # parameter folding: v_src/v_dst[f] = sum_o w[f,o] * a[o]
        vsrc = np.stack(
            [w2[0] @ a_src[heads[0], :, 0], w2[1] @ a_src[heads[1], :, 0]],
            axis=1,
        )                                                # [F, 2]
        vdst = np.stack(
            [w2[0] @ a_dst[heads[0], :, 0], w2[1] @ a_dst[heads[1], :, 0]],
            axis=1,
        )                                                # [F, 2]
        wall = np.ascontiguousarray(
            np.concatenate(
                [w2.transpose(1, 0, 2).reshape(F, NH * O),
                 0.8 * vdst, 0.2 * vdst, -0.8 * vdst, vdst],
                axis=1,
            )
        )                                                # [F, WC]
        # j-permutation: this core's own i-block columns first (they ride
        # in the hTi load), remaining j's after; adjT rows follow the same
        # permutation (j is a contraction axis, so this is value-preserving)
        perm = np.r_[i0 : i0 + NI, 0:i0, i0 + NI : N]
        in_maps.append(
            {
                "hTf": np.ascontiguousarray(hT[:, perm[NI:]]).astype(
                    ml_dtypes.bfloat16
                ),
                "hTi": np.ascontiguousarray(hT[:, i0 : i0 + NI]).astype(
                    ml_dtypes.bfloat16
                ),
                "adjT": np.ascontiguousarray(
                    adjT[perm, i0 : i0 + NI]
                    .reshape(8, 4, 128, NI)
                    .transpose(0, 2, 1, 3)
                    .reshape(8 * 128, 4 * NI)
                ),
                "wall": wall.astype(ml_dtypes.bfloat16),
                "vsrc": np.ascontiguousarray(vsrc).astype(ml_dtypes.bfloat16),
            }
        )
    return in_maps


def kernel(h, adj, w, a_src, a_dst, bias):
    from concourse.bass_utils import run_bass_kernel_spmd

    if "nc" not in _CACHE:
        _CACHE["nc"] = _build()
    nc = _CACHE["nc"]

    in_maps = _prep_inputs(h, adj, w, a_src, a_dst, bias)
    res = run_bass_kernel_spmd(nc, in_maps, list(range(NCORES))).results

    out = np.empty((N, NHEAD, O), dtype=np.float32)
    for c in range(NCORES):
        hb, ib = c % 2, c // 2
        arr = np.asarray(res[c]["out"], dtype=np.float32)  # [NH, O+1, NI]
        for hh in range(NH):
            num = arr[hh, 0:O, :]                          # [O, NI]
            den = arr[hh, O, :]                            # [NI]
            out[NI * ib : NI * (ib + 1), 2 * hb + hh, :] = (num / den).T
    out += np.asarray(bias, dtype=np.float32).reshape(1, 1, O)
    return out
